# revision 1
# baseline (speedup 1.0000x reference)
"""Classical self-attention (head-summed scores) on 8 trn2 NeuronCores.

Key algebraic rewrite: the reference sums scores over heads AND head dim,
so  S = (x Wq)(x Wk)^T / 8 = x A x^T  with A = Wq Wk^T / 8, and
    out = softmax(S) (x Wv) Wout + b = softmax(S) x W2 + b,  W2 = Wv Wout.
A and W2 are [E, E] weight-only products folded on the host, which removes
the K/V projections and the output projection from the device entirely.

Per-core math (core c = (batch b, query-half): 1024 queries, 2048 keys):
    T^T = A^T x_q^T            [E, 1024]   (the "query" projection)
    S^T[k, q] = x^T^T T^T      per 128-key tile, PSUM f32
    P = exp(S^T)               bf16, no max-subtraction (scores ~ N(0,4))
    U^T = sum_m x_m^T P^T[m]   [E, 1024]   (P x, contracted over keys)
    y = (U W2) * recip + b     natural layout, rowsums via ones-matmul

All matmuls bf16 (rate 1.0 cycles/row, same as fp32r, half the SBUF/DMA);
everything SBUF-resident — no DRAM staging round-trips.  ~393k PE rows
per core ≈ 164 us floor.

Scheduling notes:
  - T phase runs as 2 waves of 8 interleaved PSUM chains (fi outer) so PE
    consumes the (a[fi], xTq[fi]) DMA pairs as they arrive instead of
    stalling a single chain on the last pair.
  - Y phase post-processing is chunked: ACT applies the 1/rowsum scale
    straight out of PSUM (per-partition AP scale), DVE adds the bias,
    output store is bf16 — keeps the post-matmul tail short.
"""

import sys

sys.path.insert(0, "/opt/trn_rl_repo")

import numpy as np
from ml_dtypes import bfloat16

import concourse.bass as bass
import concourse.mybir as mybir
import concourse.tile as tile
from concourse import bacc

B, N, E = 4, 2048, 1024
NQ = N // 2          # query rows per core
P = 128              # partitions
FT = E // P          # 8 feature tiles
MT = N // P          # 16 key tiles
QT = NQ // P         # 8 query tiles
H = NQ // 2          # 512-column matmul halves (one PSUM bank)
F32 = mybir.dt.float32
BF16 = mybir.dt.bfloat16


def build_program():
    nc = bacc.Bacc("TRN2", target_bir_lowering=False, debug=False)
    xT = nc.dram_tensor("xT", [E, N], BF16, kind="ExternalInput").ap()
    xn = nc.dram_tensor("xn", [N, E], BF16, kind="ExternalInput").ap()
    a = nc.dram_tensor("a", [E, E], BF16, kind="ExternalInput").ap()
    w2 = nc.dram_tensor("w2", [E, E], BF16, kind="ExternalInput").ap()
    bout = nc.dram_tensor("bout", [E], BF16, kind="ExternalInput").ap()
    y = nc.dram_tensor("y", [NQ, E], BF16, kind="ExternalOutput").ap()

    with tile.TileContext(nc) as tc:
        _body(nc, tc, xT, xn, a, w2, bout, y)
    nc.compile()
    return nc


def _body(nc, tc, xT, xn, a, w2, bout, y):
    cst = tc.alloc_tile_pool(name="cst", bufs=1)
    ones = cst.tile([P, 1], BF16, name="ones", tag="ones")
    nc.vector.memset(ones, 1.0)
    warm = cst.tile([P, 256], BF16, name="warm", tag="warm")
    nc.vector.memset(warm, 0.0)
    # DMA issue order tracks first-use order: the T projection consumes
    # (a[fi], xTq[fi]) pairs in fi order, so those two queues interleave;
    # xTk feeds the back half of the S phase, xn the U phase, w2/bout the
    # Y phase.
    a_t, xTq_t = [], []
    for f in range(FT):
        at = cst.tile([P, E], BF16, name=f"a{f}", tag=f"a{f}")
        # First two a tiles ride the scalar engine's HWDGE (fast fixed
        # overhead, idle at start) so wave 1 gets its first pair sooner.
        eng = nc.scalar if f < 2 else nc.gpsimd
        eng.dma_start(out=at, in_=a[f * P:(f + 1) * P, :])
        a_t.append(at)
        xq = cst.tile([P, NQ], BF16, name=f"xTq{f}", tag=f"xTq{f}")
        nc.sync.dma_start(out=xq, in_=xT[f * P:(f + 1) * P, 0:NQ])
        xTq_t.append(xq)
    xTk_t = []
    for f in range(FT):
        xk = cst.tile([P, NQ], BF16, name=f"xTk{f}", tag=f"xTk{f}")
        nc.sync.dma_start(out=xk, in_=xT[f * P:(f + 1) * P, NQ:N])
        xTk_t.append(xk)
    xn_t = [cst.tile([P, E], BF16, name=f"xn{m}", tag=f"xn{m}")
            for m in range(MT)]
    for m in range(MT):
        nc.gpsimd.dma_start(out=xn_t[m], in_=xn[m * P:(m + 1) * P, :])
    w2_t = [cst.tile([P, E], BF16, name=f"w2{f}", tag=f"w2{f}")
            for f in range(FT)]
    for f in range(FT):
        nc.gpsimd.dma_start(out=w2_t[f], in_=w2[f * P:(f + 1) * P, :])
    bo_b = cst.tile([P, E], BF16, name="bo_b", tag="bo_b")
    bout_bcast = bass.AP(tensor=bout.tensor, offset=0, ap=[[0, P], [1, E]])
    nc.sync.dma_start(out=bo_b, in_=bout_bcast)

    tT_p = tc.alloc_tile_pool(name="tTp", bufs=1)
    tT_t = [tT_p.tile([P, NQ], BF16, name=f"tT{f}", tag=f"tT{f}")
            for f in range(FT)]

    # Warm the PE pstate ramp (~3us of dummy matmuls) while the first input
    # DMAs land, so the real chains start at full clock.
    with tc.tile_pool(name="wps", bufs=1, space="PSUM") as wpp:
        wps = wpp.tile([P, 256], F32, name="wps", tag="wps")
        for i in range(14):
            nc.tensor.matmul(wps, warm[:, 0:P], warm, start=True, stop=True)

    # ---- T^T = A^T x_q^T: 2 waves x 8 chains, fi outer so each DMA pair
    # unblocks one matmul step of every open chain ----
    with tc.tile_pool(name="tps", bufs=8, space="PSUM") as tpp:
        # Wave 1: 8 chains interleaved fi-outer, so each (a[fi], xTq[fi])
        # DMA pair unblocks one step of every open chain while loads land.
        pss = [tpp.tile([P, H], F32, name=f"tp{c}", tag="tp")
               for c in range(8)]
        for fi in range(FT):
            for c in range(8):
                fo, h = c // 2, c % 2
                nc.tensor.matmul(
                    pss[c], a_t[fi][:, fo * P:(fo + 1) * P],
                    xTq_t[fi][:, h * H:(h + 1) * H],
                    start=(fi == 0), stop=(fi == FT - 1))
        for c in range(8):
            fo, h = c // 2, c % 2
            nc.vector.tensor_copy(tT_t[fo][:, h * H:(h + 1) * H], pss[c])
        # Wave 2: inputs all present by now; sequential chains so the
        # PSUM->SBUF copies spread out instead of bunching before S starts.
        for c in range(8):
            fo, h = 4 + c // 2, c % 2
            ps = tpp.tile([P, H], F32, name=f"tp2{c}", tag="tp")
            for fi in range(FT):
                nc.tensor.matmul(
                    ps, a_t[fi][:, fo * P:(fo + 1) * P],
                    xTq_t[fi][:, h * H:(h + 1) * H],
                    start=(fi == 0), stop=(fi == FT - 1))
            nc.vector.tensor_copy(tT_t[fo][:, h * H:(h + 1) * H], ps)

    # ---- S^T per key tile; P = exp(S^T); rowsums lag one tile ----
    recp = tc.alloc_tile_pool(name="recp", bufs=1, side="right")
    pres = tc.alloc_tile_pool(name="pres", bufs=1)
    smp = tc.alloc_tile_pool(name="smp", bufs=1, side="right")
    sums_acc = smp.tile([P, QT], F32, name="sums_acc", tag="sums_acc")
    uT_p = tc.alloc_tile_pool(name="uTp", bufs=1, side="right")
    uT_t = [uT_p.tile([P, NQ], BF16, name=f"uT{f}", tag=f"uT{f}")
            for f in range(FT)]
    p_t = []
    # S and U PSUM pools share one scope (4 + 2 + 2 = 8 banks), so the U
    # chains start the moment p[0] exists instead of waiting for the whole
    # S-pool range to release.
    with tc.tile_pool(name="sps", bufs=2, space="PSUM") as spp, \
         tc.tile_pool(name="sums", bufs=2, space="PSUM") as sumsp, \
         tc.tile_pool(name="ups", bufs=2, space="PSUM") as upp:
        for m in range(MT):
            xcol = xTq_t if m < QT else xTk_t
            mm = m % QT
            s = spp.tile([P, NQ], F32, name="s", tag="s")
            for f in range(FT):
                for h in range(2):
                    nc.tensor.matmul(
                        s[:, h * H:(h + 1) * H],
                        xcol[f][:, mm * P:(mm + 1) * P],
                        tT_t[f][:, h * H:(h + 1) * H],
                        start=(f == 0), stop=(f == FT - 1))
            p = pres.tile([P, NQ], BF16, name=f"p{m}", tag=f"p{m}")
            nc.scalar.activation(p, s, mybir.ActivationFunctionType.Exp)
            p_t.append(p)
            if m > 0:
                _row_sums(nc, p_t[m - 1], sumsp, ones, sums_acc,
                          first=(m == 1))
        _row_sums(nc, p_t[MT - 1], sumsp, ones, sums_acc, first=False)
        recip = recp.tile([P, QT], F32, name="recip", tag="recip")
        nc.vector.reciprocal(recip, sums_acc)

        # ---- U^T[f] = sum_m xn[m][:, f]^T P^T[m], half-tile chains ----
        for fo in range(FT):
            for hh in range(2):
                u_ps = upp.tile([P, H], F32, name="u", tag="u")
                for m in range(MT):
                    nc.tensor.matmul(
                        u_ps,
                        xn_t[m][:, fo * P:(fo + 1) * P],
                        p_t[m][:, hh * H:(hh + 1) * H],
                        start=(m == 0), stop=(m == MT - 1))
                nc.vector.tensor_copy(uT_t[fo][:, hh * H:(hh + 1) * H], u_ps)

    # ---- y = (U W2) * recip + b, natural [q, e] layout ----
    with tc.tile_pool(name="yps", bufs=4, space="PSUM") as ypp, \
         tc.tile_pool(name="ysb", bufs=8) as ysp:
        for qt in range(QT):
            for h in range(2):
                # Per-(qt, half) chains: each half's post-processing
                # overlaps the next chain, so only one 512-col post chain
                # trails the final matmul.
                yps = ypp.tile([P, H], F32, name=f"yps{h}", tag="yps")
                for f in range(FT):
                    nc.tensor.matmul(
                        yps,
                        uT_t[f][:, qt * P:(qt + 1) * P],
                        w2_t[f][:, h * H:(h + 1) * H],
                        start=(f == 0), stop=(f == FT - 1))
                ysb = ysp.tile([P, H], BF16, name="ysb", tag="ysb")
                nc.scalar.activation(ysb, yps,
                                     mybir.ActivationFunctionType.Copy,
                                     scale=recip[:, qt:qt + 1])
                nc.vector.tensor_tensor(out=ysb, in0=ysb,
                                        in1=bo_b[:, h * H:(h + 1) * H],
                                        op=mybir.AluOpType.add)
                # Alternate DGE paths so consecutive stores never queue
                # behind one HWDGE grab.
                eng = nc.gpsimd if h == 0 else nc.sync
                eng.dma_start(
                    out=y[qt * P:(qt + 1) * P, h * H:(h + 1) * H], in_=ysb)

    pres.release()
    uT_p.release()
    smp.release()
    recp.release()
    tT_p.release()
    cst.release()


def _row_sums(nc, p, sumsp, ones, sums_acc, first):
    sums_m = sumsp.tile([P, QT], F32, name="sums_m", tag="sums_m")
    for q in range(QT):
        nc.tensor.matmul(sums_m[:, q:q + 1], p[:, q * P:(q + 1) * P], ones,
                         start=True, stop=True)
    if first:
        nc.vector.tensor_copy(sums_acc, sums_m)
    else:
        nc.vector.tensor_tensor(out=sums_acc, in0=sums_acc,
                                in1=sums_m, op=mybir.AluOpType.add)


_NC_CACHE = None


def _get_program():
    global _NC_CACHE
    if _NC_CACHE is None:
        _NC_CACHE = build_program()
    return _NC_CACHE


def _host_prep(x, W_qkv, W_out, b_out):
    """Fold weights and build the per-core input maps."""
    Wq = W_qkv[:, :E]
    Wk = W_qkv[:, E:2 * E]
    Wv = W_qkv[:, 2 * E:]
    A = ((Wq @ Wk.T) * 0.125).astype(bfloat16)
    W2 = (Wv @ W_out).astype(bfloat16)
    bo = b_out.astype(bfloat16)
    in_maps = []
    for c in range(8):
        b, half = divmod(c, 2)
        xb = x[b]
        # Rotate so this core's 1024 query rows come first; key order is
        # irrelevant (softmax sums over all keys).
        xrot = np.concatenate([xb[half * NQ:], xb[:half * NQ]], axis=0)
        xrot_bf = xrot.astype(bfloat16)
        in_maps.append({
            "xT": np.ascontiguousarray(xrot_bf.T),
            "xn": xrot_bf,
            "a": A,
            "w2": W2,
            "bout": bo,
        })
    return in_maps


def kernel(x, W_qkv, W_out, b_out):
    from concourse.bass_utils import run_bass_kernel_spmd

    x = np.asarray(x, dtype=np.float32)
    W_qkv = np.asarray(W_qkv, dtype=np.float32)
    W_out = np.asarray(W_out, dtype=np.float32)
    b_out = np.asarray(b_out, dtype=np.float32)

    nc = _get_program()
    in_maps = _host_prep(x, W_qkv, W_out, b_out)
    res = run_bass_kernel_spmd(nc, in_maps, list(range(8)))
    out = np.empty((B, N, E), dtype=np.float32)
    for c in range(8):
        b, half = divmod(c, 2)
        out[b, half * NQ:(half + 1) * NQ] = res.results[c]["y"].astype(
            np.float32)
    return out



# revision 5
# speedup vs baseline: 1.0870x; 1.0870x over previous
"""Classical self-attention (head-summed scores) on 8 trn2 NeuronCores.

Algebraic rewrite (as baseline): scores sum over heads AND head dim, so
  S = x A x^T with A = Wq Wk^T / 8,  out = softmax(S) x W2 + b, W2 = Wv Wout.
A and W2 are folded on the host.

v2: every matmul phase runs as fp8e4m3 DoubleRow (perf_mode) with hi/lo
splitting.  Each operand v is stored as v_hi = e4m3(v), v_lo = e4m3(v - v_hi);
a product contracts 256 elements per instruction (pair-packed along the
contraction dim) via three term-matmuls (hi*hi, hi*lo, lo*hi), costing
1.5 x 0.5 = 0.75 cycles/row-of-128 vs bf16's 1.0 -> ~123us PE floor
(294912 rows) instead of 164us.  Measured end-to-end rel err ~5e-3
(better than the bf16 baseline's 8.4e-3: the e4m3 hi+lo pair carries ~9
mantissa bits).

Ranges (e4m3 max normal is 240, overflow -> inf, so everything is scaled):
  A8 = 128*A   (std ~0.5), x8 = x (std 1), T_psum = 128*T (std 16)
  T8 = split8(T_psum) directly from PSUM, exp scale 1/128
  P = exp(S/1) bf16 unscaled; r = 32/rowsum(P) per query -> p8 = split8(P*r)
  (attention weights <= 1 so P*r <= 32; per-query scale cancels because the
  final normalizer is recomputed from p8 itself)
  U_psum = U*r (<=176), U8 = split8(U_psum); W28 = split8(16*W2)
  out = Y_psum / (16 * rowsum(p8)) + b

Schedule: S and U are split by query halves h in {0,1} (512 cols each).
While PE runs S(h=1), the h=0 pipeline (rowsums -> r -> PE-transpose ->
DRAM bounce -> broadcast row -> p8 tiles) runs on ACT/DVE/DMA; U(hh=0)
consumes finished p8 h0 tiles while the h1 pipeline fills, and Y(qt 0-3)
overlaps U(hh=1).  The only PE instructions off the main chain are two
128-cycle transposes and 256 one-row rowsum matmuls.
"""

import sys

sys.path.insert(0, "/opt/trn_rl_repo")

import numpy as np
import ml_dtypes
from ml_dtypes import bfloat16

E4NP = ml_dtypes.float8_e4m3

import concourse.bass as bass
import concourse.mybir as mybir
import concourse.tile as tile
from concourse import bacc

B, N, E = 4, 2048, 1024
NQ = N // 2          # query rows per core
P = 128              # partitions
FP = 4               # e-tile pairs (contraction E = 4 * 256)
MT = 16              # key tiles
MP = 8               # key tile pairs
QT = 8               # query blocks of 128
H = NQ // 2          # 512-column matmul halves (one PSUM bank)
F32 = mybir.dt.float32
BF16 = mybir.dt.bfloat16
F8 = mybir.dt.float8e4
DR = mybir.MatmulPerfMode.DoubleRow
EXP_SCALE = 1.0 / 128.0
R_SCALE = 32.0
RECF_SCALE = 16.0    # W28 pre-scale folded out of the final reciprocal


def build_program():
    nc = bacc.Bacc("TRN2", target_bir_lowering=False, debug=False)
    a8h = nc.dram_tensor("a8h", [FP, P, 2, E], F8, kind="ExternalInput").ap()
    a8l = nc.dram_tensor("a8l", [FP, P, 2, E], F8, kind="ExternalInput").ap()
    xq8h = nc.dram_tensor("xq8h", [FP, P, 2, NQ], F8, kind="ExternalInput").ap()
    xq8l = nc.dram_tensor("xq8l", [FP, P, 2, NQ], F8, kind="ExternalInput").ap()
    xk8h = nc.dram_tensor("xk8h", [FP, P, 2, NQ], F8, kind="ExternalInput").ap()
    xk8l = nc.dram_tensor("xk8l", [FP, P, 2, NQ], F8, kind="ExternalInput").ap()
    xn8h = nc.dram_tensor("xn8h", [MP, P, 2, E], F8, kind="ExternalInput").ap()
    xn8l = nc.dram_tensor("xn8l", [MP, P, 2, E], F8, kind="ExternalInput").ap()
    w28h = nc.dram_tensor("w28h", [FP, P, 2, E], F8, kind="ExternalInput").ap()
    w28l = nc.dram_tensor("w28l", [FP, P, 2, E], F8, kind="ExternalInput").ap()
    bout = nc.dram_tensor("bout", [E], BF16, kind="ExternalInput").ap()
    ident = nc.dram_tensor("ident", [P, P], BF16, kind="ExternalInput").ap()
    rsc = nc.dram_tensor("rsc", [2, 4, P], BF16, kind="Internal").ap()
    y = nc.dram_tensor("y", [NQ, E], BF16, kind="ExternalOutput").ap()

    with tile.TileContext(nc) as tc:
        _body(nc, tc, a8h, a8l, xq8h, xq8l, xk8h, xk8l, xn8h, xn8l,
              w28h, w28l, bout, ident, rsc, y)
    nc.compile()
    return nc


def _body(nc, tc, a8h, a8l, xq8h, xq8l, xk8h, xk8l, xn8h, xn8l,
          w28h, w28l, bout, ident, rsc, y):
    Exp = mybir.ActivationFunctionType.Exp
    Copy = mybir.ActivationFunctionType.Copy
    Add = mybir.AluOpType.add
    Sub = mybir.AluOpType.subtract
    Mul = mybir.AluOpType.mult

    cst = tc.alloc_tile_pool(name="cst", bufs=1)
    ones = cst.tile([P, 1], BF16, name="ones", tag="ones")
    nc.vector.memset(ones, 1.0)
    ones8 = cst.tile([P, 2, 1], F8, name="ones8", tag="ones8")
    nc.vector.memset(ones8, 1.0)
    warm = cst.tile([P, 256], BF16, name="warm", tag="warm")
    nc.vector.memset(warm, 0.0)

    # ---- input loads; issue order tracks first-use order ----
    t8p = tc.alloc_tile_pool(name="t8p", bufs=1)
    T8h_t = [t8p.tile([P, 2, NQ], F8, name=f"T8h{f}", tag=f"T8h{f}")
             for f in range(FP)]
    T8l_t = [t8p.tile([P, 2, NQ], F8, name=f"T8l{f}", tag=f"T8l{f}")
             for f in range(FP)]

    pa = tc.alloc_tile_pool(name="pa", bufs=1)
    a8h_t, a8l_t, xq8h_t, xq8l_t = [], [], [], []
    for f in range(FP):
        ah = pa.tile([P, 2, E], F8, name=f"a8h{f}", tag=f"a8h{f}")
        qh = cst.tile([P, 2, NQ], F8, name=f"xq8h{f}", tag=f"xq8h{f}")
        al = pa.tile([P, 2, E], F8, name=f"a8l{f}", tag=f"a8l{f}")
        ql = cst.tile([P, 2, NQ], F8, name=f"xq8l{f}", tag=f"xq8l{f}")
        # first fp pair rides the scalar HWDGE so wave 1 starts sooner
        eng = nc.scalar if f == 0 else nc.sync
        eng.dma_start(out=ah, in_=a8h[f])
        eng.dma_start(out=qh, in_=xq8h[f])
        eng.dma_start(out=al, in_=a8l[f])
        eng.dma_start(out=ql, in_=xq8l[f])
        a8h_t.append(ah); a8l_t.append(al)
        xq8h_t.append(qh); xq8l_t.append(ql)
    ident_t = cst.tile([P, P], BF16, name="ident_t", tag="ident_t")
    nc.scalar.dma_start(out=ident_t, in_=ident)
    bo_b = cst.tile([P, E], BF16, name="bo_b", tag="bo_b")
    bout_bcast = bass.AP(tensor=bout.tensor, offset=0, ap=[[0, P], [1, E]])
    nc.scalar.dma_start(out=bo_b, in_=bout_bcast)
    xk8h_t, xk8l_t = [], []
    for f in range(FP):
        kh = cst.tile([P, 2, NQ], F8, name=f"xk8h{f}", tag=f"xk8h{f}")
        kl = cst.tile([P, 2, NQ], F8, name=f"xk8l{f}", tag=f"xk8l{f}")
        nc.gpsimd.dma_start(out=kh, in_=xk8h[f])
        nc.gpsimd.dma_start(out=kl, in_=xk8l[f])
        xk8h_t.append(kh); xk8l_t.append(kl)
    xn8h_t, xn8l_t = [], []
    for m in range(MP):
        nh = cst.tile([P, 2, E], F8, name=f"xn8h{m}", tag=f"xn8h{m}")
        nl = cst.tile([P, 2, E], F8, name=f"xn8l{m}", tag=f"xn8l{m}")
        nc.gpsimd.dma_start(out=nh, in_=xn8h[m])
        nc.gpsimd.dma_start(out=nl, in_=xn8l[m])
        xn8h_t.append(nh); xn8l_t.append(nl)
    w28h_t, w28l_t = [], []
    for f in range(FP):
        wh = cst.tile([P, 2, E], F8, name=f"w28h{f}", tag=f"w28h{f}")
        wl = cst.tile([P, 2, E], F8, name=f"w28l{f}", tag=f"w28l{f}")
        nc.gpsimd.dma_start(out=wh, in_=w28h[f])
        nc.gpsimd.dma_start(out=wl, in_=w28l[f])
        w28h_t.append(wh); w28l_t.append(wl)

    # ---- PE pstate warmup while first loads land ----
    with tc.tile_pool(name="wps", bufs=1, space="PSUM") as wpp:
        wps = wpp.tile([P, 256], F32, name="wps", tag="wps")
        for i in range(14):
            nc.tensor.matmul(wps, warm[:, 0:P], warm, start=True, stop=True)

    def terms(wh, wl, rh, rl):
        return ((wh, rh), (wh, rl), (wl, rh))

    # ---- T^T = (128 A)^T x_q^T : 2 waves x 8 interleaved chains ----
    with tc.tile_pool(name="tps", bufs=8, space="PSUM") as tpp:
        pss = [tpp.tile([P, H], F32, name=f"tp{c}", tag="tp")
               for c in range(8)]
        for fp in range(FP):
            for c in range(8):
                fo, h = c // 2, c % 2
                for t, (wa, rb) in enumerate(
                        terms(a8h_t[fp], a8l_t[fp], xq8h_t[fp], xq8l_t[fp])):
                    nc.tensor.matmul(
                        pss[c], wa[:, :, fo * P:(fo + 1) * P],
                        rb[:, :, h * H:(h + 1) * H],
                        start=(fp == 0 and t == 0),
                        stop=(fp == FP - 1 and t == 2), perf_mode=DR)
        for c in range(8):
            fo, h = c // 2, c % 2
            hs = T8h_t[fo // 2][:, fo % 2, h * H:(h + 1) * H]
            nc.scalar.activation(hs, pss[c], Copy)
            nc.vector.tensor_tensor(
                out=T8l_t[fo // 2][:, fo % 2, h * H:(h + 1) * H],
                in0=pss[c], in1=hs, op=Sub)
        for c in range(8):
            fo, h = 4 + c // 2, c % 2
            ps = tpp.tile([P, H], F32, name=f"tp2{c}", tag="tp")
            for fp in range(FP):
                for t, (wa, rb) in enumerate(
                        terms(a8h_t[fp], a8l_t[fp], xq8h_t[fp], xq8l_t[fp])):
                    nc.tensor.matmul(
                        ps, wa[:, :, fo * P:(fo + 1) * P],
                        rb[:, :, h * H:(h + 1) * H],
                        start=(fp == 0 and t == 0),
                        stop=(fp == FP - 1 and t == 2), perf_mode=DR)
            hs = T8h_t[fo // 2][:, fo % 2, h * H:(h + 1) * H]
            nc.scalar.activation(hs, ps, Copy)
            nc.vector.tensor_tensor(
                out=T8l_t[fo // 2][:, fo % 2, h * H:(h + 1) * H],
                in0=ps, in1=hs, op=Sub)

    pa.release()

    # ---- S, softmax pipeline, U, Y ----
    pres = tc.alloc_tile_pool(name="pres", bufs=1)
    p_t = [pres.tile([P, NQ], BF16, name=f"p{m}", tag=f"p{m}")
           for m in range(MT)]
    p8p = tc.alloc_tile_pool(name="p8p", bufs=1, side="right")
    p8h_t = [p8p.tile([P, 2, NQ], F8, name=f"p8h{m}", tag=f"p8h{m}")
             for m in range(MP)]
    p8l_t = [p8p.tile([P, 2, NQ], F8, name=f"p8l{m}", tag=f"p8l{m}")
             for m in range(MP)]
    u8p = tc.alloc_tile_pool(name="u8p", bufs=1, side="right")
    U8h_t = [u8p.tile([P, 2, NQ], F8, name=f"U8h{f}", tag=f"U8h{f}")
             for f in range(FP)]
    U8l_t = [u8p.tile([P, 2, NQ], F8, name=f"U8l{f}", tag=f"U8l{f}")
             for f in range(FP)]
    smp = tc.alloc_tile_pool(name="smp", bufs=1, side="right")
    sums_acc = [smp.tile([P, 4], F32, name=f"sums_acc{h}", tag=f"sums_acc{h}")
                for h in range(2)]
    r8_acc = [smp.tile([P, 4], F32, name=f"r8_acc{h}", tag=f"r8_acc{h}")
              for h in range(2)]
    recf = [smp.tile([P, 4], F32, name=f"recf{h}", tag=f"recf{h}")
            for h in range(2)]
    rrow = [smp.tile([4, P], BF16, name=f"rrow{h}", tag=f"rrow{h}")
            for h in range(2)]
    r_rep = [smp.tile([P, H], BF16, name=f"r_rep{h}", tag=f"r_rep{h}")
             for h in range(2)]
    tmpp = tc.alloc_tile_pool(name="tmpp", bufs=4, side="right")

    def s_chain(h, mt, spp):
        xsh = xq8h_t if mt < 8 else xk8h_t
        xsl = xq8l_t if mt < 8 else xk8l_t
        mm = mt % 8
        s = spp.tile([P, H], F32, name="s", tag="s")
        for fp in range(FP):
            for t, (wa, rb) in enumerate(
                    terms(xsh[fp], xsl[fp], T8h_t[fp], T8l_t[fp])):
                nc.tensor.matmul(
                    s, wa[:, :, mm * P:(mm + 1) * P],
                    rb[:, :, h * H:(h + 1) * H],
                    start=(fp == 0 and t == 0),
                    stop=(fp == FP - 1 and t == 2), perf_mode=DR)
        nc.scalar.activation(p_t[mt][:, h * H:(h + 1) * H], s, Exp,
                             scale=EXP_SCALE)

    def row_sums(h, mt, sumsp):
        sm = sumsp.tile([P, 4], F32, name="sums_m", tag="sums_m")
        for q in range(4):
            nc.tensor.matmul(
                sm[:, q:q + 1],
                p_t[mt][:, h * H + q * P: h * H + (q + 1) * P], ones,
                start=True, stop=True)
        if mt == 0:
            nc.vector.tensor_copy(sums_acc[h], sm)
        else:
            nc.vector.tensor_tensor(out=sums_acc[h], in0=sums_acc[h],
                                    in1=sm, op=Add)

    def r_path(h, rtpp):
        """sums_acc[h] -> r_rep[h] broadcast row (32/rowsum as bf16)."""
        rf = tmpp.tile([P, 4], F32, name="rf", tag="rf")
        nc.vector.reciprocal(rf, sums_acc[h])
        rb16 = tmpp.tile([P, 4], BF16, name="rb16", tag="rb16")
        nc.vector.tensor_scalar_mul(rb16, rf, R_SCALE)
        rtp = rtpp.tile([4, P], BF16, name="rtp", tag="rtp")
        nc.tensor.transpose(rtp, rb16, ident_t)
        nc.vector.tensor_copy(rrow[h], rtp)
        nc.sync.dma_start(out=rsc[h], in_=rrow[h])
        rsc_b = bass.AP(tensor=rsc.tensor, offset=h * 4 * P,
                        ap=[[0, P], [1, H]])
        nc.sync.dma_start(out=r_rep[h], in_=rsc_b)

    def p8_prod(h, mt):
        mp, i = mt // 2, mt % 2
        tmp = tmpp.tile([P, H], BF16, name="tmp", tag="tmp")
        nc.vector.tensor_tensor(out=tmp, in0=p_t[mt][:, h * H:(h + 1) * H],
                                in1=r_rep[h], op=Mul)
        hs = p8h_t[mp][:, i, h * H:(h + 1) * H]
        nc.scalar.activation(hs, tmp, Copy)
        nc.vector.tensor_tensor(out=p8l_t[mp][:, i, h * H:(h + 1) * H],
                                in0=tmp, in1=hs, op=Sub)

    def u_chain(hh, fo, upp):
        u = upp.tile([P, H], F32, name="u", tag="u")
        for mp in range(MP):
            for t, (wa, rb) in enumerate(
                    terms(xn8h_t[mp], xn8l_t[mp], p8h_t[mp], p8l_t[mp])):
                nc.tensor.matmul(
                    u, wa[:, :, fo * P:(fo + 1) * P],
                    rb[:, :, hh * H:(hh + 1) * H],
                    start=(mp == 0 and t == 0),
                    stop=(mp == MP - 1 and t == 2), perf_mode=DR)
        hs = U8h_t[fo // 2][:, fo % 2, hh * H:(hh + 1) * H]
        nc.scalar.activation(hs, u, Copy)
        nc.vector.tensor_tensor(
            out=U8l_t[fo // 2][:, fo % 2, hh * H:(hh + 1) * H],
            in0=u, in1=hs, op=Sub)

    def r8_sums(hh, mp, sumsp):
        """rowsums of p8 (hi+lo) for query blocks of half hh."""
        for j, px in enumerate((p8h_t, p8l_t)):
            sm = sumsp.tile([P, 4], F32, name="r8_m", tag="sums_m")
            for q in range(4):
                nc.tensor.matmul(
                    sm[:, q:q + 1],
                    px[mp][:, :, hh * H + q * P: hh * H + (q + 1) * P],
                    ones8, start=True, stop=True, perf_mode=DR)
            if mp == 0 and j == 0:
                nc.vector.tensor_copy(r8_acc[hh], sm)
            else:
                nc.vector.tensor_tensor(out=r8_acc[hh], in0=r8_acc[hh],
                                        in1=sm, op=Add)

    def recf_path(hh):
        r8s = tmpp.tile([P, 4], F32, name="r8s", tag="rf")
        nc.vector.tensor_scalar_mul(r8s, r8_acc[hh], RECF_SCALE)
        nc.vector.reciprocal(recf[hh], r8s)

    def y_chain(qt, he, ypp, ysp):
        hh = qt // 4
        yps = ypp.tile([P, H], F32, name="yps", tag="yps")
        for fp in range(FP):
            for t, (wa, rb) in enumerate(
                    terms(U8h_t[fp], U8l_t[fp], w28h_t[fp], w28l_t[fp])):
                nc.tensor.matmul(
                    yps, wa[:, :, qt * P:(qt + 1) * P],
                    rb[:, :, he * H:(he + 1) * H],
                    start=(fp == 0 and t == 0),
                    stop=(fp == FP - 1 and t == 2), perf_mode=DR)
        ysb = ysp.tile([P, H], BF16, name="ysb", tag="ysb")
        nc.scalar.activation(ysb, yps, Copy,
                             scale=recf[hh][:, (qt % 4):(qt % 4) + 1])
        nc.vector.tensor_tensor(out=ysb, in0=ysb,
                                in1=bo_b[:, he * H:(he + 1) * H], op=Add)
        eng = nc.gpsimd if he == 0 else nc.sync
        eng.dma_start(out=y[qt * P:(qt + 1) * P, he * H:(he + 1) * H],
                      in_=ysb)

    with tc.tile_pool(name="smps", bufs=2, space="PSUM") as sumsp, \
         tc.tile_pool(name="rtps", bufs=2, space="PSUM") as rtpp:
        with tc.tile_pool(name="sps", bufs=2, space="PSUM") as spp:
            # S half 0
            for mt in range(MT):
                s_chain(0, mt, spp)
                row_sums(0, mt, sumsp)
            # S half 1, with the h0 softmax pipeline interleaved
            s_chain(1, 0, spp)
            row_sums(1, 0, sumsp)
            r_path(0, rtpp)
            for mt in range(1, MT):
                s_chain(1, mt, spp)
                row_sums(1, mt, sumsp)
                p8_prod(0, mt - 1)
            p8_prod(0, MT - 1)
        with tc.tile_pool(name="ups", bufs=2, space="PSUM") as upp:
            # U half 0; h1 softmax pipeline + h0 fp8 rowsums interleaved
            u_chain(0, 0, upp)
            r_path(1, rtpp)
            for fo in range(1, 8):
                u_chain(0, fo, upp)
                p8_prod(1, 2 * (fo - 1))
                p8_prod(1, 2 * (fo - 1) + 1)
                r8_sums(0, fo - 1, sumsp)
            p8_prod(1, MT - 2)
            p8_prod(1, MT - 1)
            r8_sums(0, MP - 1, sumsp)
            recf_path(0)
            # U half 1; h1 fp8 rowsums interleaved
            for fo in range(8):
                u_chain(1, fo, upp)
                if fo < MP:
                    r8_sums(1, fo, sumsp)
            recf_path(1)
        with tc.tile_pool(name="yps_p", bufs=4, space="PSUM") as ypp, \
             tc.tile_pool(name="ysb_p", bufs=4) as ysp:
            for qt in (0, 1, 2, 3):
                for he in range(2):
                    y_chain(qt, he, ypp, ysp)
            for qt in (4, 5, 6, 7):
                for he in range(2):
                    y_chain(qt, he, ypp, ysp)

    tmpp.release()
    smp.release()
    u8p.release()
    p8p.release()
    pres.release()
    t8p.release()
    cst.release()


_NC_CACHE = None


def _get_program():
    global _NC_CACHE
    if _NC_CACHE is None:
        _NC_CACHE = build_program()
    return _NC_CACHE


def _split8(v):
    hi = v.astype(E4NP)
    lo = (v - hi.astype(np.float32)).astype(E4NP)
    return hi, lo


def _pack_fp(mat, cols):
    """[E, cols] -> [FP, P, 2, cols] pair-packed along the contraction dim."""
    return np.ascontiguousarray(
        mat.reshape(FP, 2, P, cols).transpose(0, 2, 1, 3))


def _host_prep(x, W_qkv, W_out, b_out):
    Wq = W_qkv[:, :E].astype(np.float64)
    Wk = W_qkv[:, E:2 * E].astype(np.float64)
    Wv = W_qkv[:, 2 * E:].astype(np.float64)
    A = ((Wq @ Wk.T) * (0.125 * 128.0)).astype(np.float32)
    W2 = ((Wv @ W_out.astype(np.float64)) * 16.0).astype(np.float32)
    A8h, A8l = _split8(A)
    W28h, W28l = _split8(W2)
    a8h_p, a8l_p = _pack_fp(A8h, E), _pack_fp(A8l, E)
    w28h_p, w28l_p = _pack_fp(W28h, E), _pack_fp(W28l, E)
    bo = b_out.astype(bfloat16)
    identity = np.eye(P, dtype=bfloat16)

    in_maps = []
    for c in range(8):
        b, half = divmod(c, 2)
        xb = np.asarray(x[b], dtype=np.float32)
        xrot = np.concatenate([xb[half * NQ:], xb[:half * NQ]], axis=0)
        x8h, x8l = _split8(xrot)
        xt8h = _pack_fp(np.ascontiguousarray(x8h.T), N)
        xt8l = _pack_fp(np.ascontiguousarray(x8l.T), N)
        xn8h = np.ascontiguousarray(
            x8h.reshape(MP, 2, P, E).transpose(0, 2, 1, 3))
        xn8l = np.ascontiguousarray(
            x8l.reshape(MP, 2, P, E).transpose(0, 2, 1, 3))
        in_maps.append({
            "a8h": a8h_p, "a8l": a8l_p,
            "xq8h": np.ascontiguousarray(xt8h[:, :, :, 0:NQ]),
            "xq8l": np.ascontiguousarray(xt8l[:, :, :, 0:NQ]),
            "xk8h": np.ascontiguousarray(xt8h[:, :, :, NQ:N]),
            "xk8l": np.ascontiguousarray(xt8l[:, :, :, NQ:N]),
            "xn8h": xn8h, "xn8l": xn8l,
            "w28h": w28h_p, "w28l": w28l_p,
            "bout": bo, "ident": identity,
        })
    return in_maps


def kernel(x, W_qkv, W_out, b_out):
    from concourse.bass_utils import run_bass_kernel_spmd

    x = np.asarray(x, dtype=np.float32)
    W_qkv = np.asarray(W_qkv, dtype=np.float32)
    W_out = np.asarray(W_out, dtype=np.float32)
    b_out = np.asarray(b_out, dtype=np.float32)

    nc = _get_program()
    in_maps = _host_prep(x, W_qkv, W_out, b_out)
    res = run_bass_kernel_spmd(nc, in_maps, list(range(8)))
    out = np.empty((B, N, E), dtype=np.float32)
    for c in range(8):
        b, half = divmod(c, 2)
        out[b, half * NQ:(half + 1) * NQ] = res.results[c]["y"].astype(
            np.float32)
    return out


# revision 18
# speedup vs baseline: 1.2316x; 1.1330x over previous
"""Classical self-attention (head-summed scores) on 8 trn2 NeuronCores.

Algebraic rewrite (as baseline): scores sum over heads AND head dim, so
  S = x A x^T with A = Wq Wk^T / 8,  out = softmax(S) x W2 + b, W2 = Wv Wout.
A and W2 are folded on the host.

v2: every matmul phase runs as fp8e4m3 DoubleRow (perf_mode) with hi/lo
splitting.  Each operand v is stored as v_hi = e4m3(v), v_lo = e4m3(v - v_hi);
a product contracts 256 elements per instruction (pair-packed along the
contraction dim) via three term-matmuls (hi*hi, hi*lo, lo*hi), costing
1.5 x 0.5 = 0.75 cycles/row-of-128 vs bf16's 1.0 -> ~123us PE floor
(294912 rows) instead of 164us.  Measured end-to-end rel err ~5e-3
(better than the bf16 baseline's 8.4e-3: the e4m3 hi+lo pair carries ~9
mantissa bits).

Ranges (e4m3 max normal is 240, overflow -> inf, so everything is scaled):
  A8 = 128*A   (std ~0.5), x8 = x (std 1), T_psum = 128*T (std 16)
  T8 = split8(T_psum) directly from PSUM, exp scale 1/128
  P = exp(S/1) bf16 unscaled; r = 32/rowsum(P) per query -> p8 = split8(P*r)
  (attention weights <= 1 so P*r <= 32; per-query scale cancels because the
  final normalizer is recomputed from p8 itself)
  U_psum = U*r (<=176), U8 = split8(U_psum); W28 = split8(16*W2)
  out = Y_psum / (16 * rowsum(p8)) + b

Schedule: S and U are split by query halves h in {0,1} (512 cols each).
While PE runs S(h=1), the h=0 pipeline (rowsums -> r -> PE-transpose ->
DRAM bounce -> broadcast row -> p8 tiles) runs on ACT/DVE/DMA; U(hh=0)
consumes finished p8 h0 tiles while the h1 pipeline fills, and Y(qt 0-3)
overlaps U(hh=1).  The only PE instructions off the main chain are two
128-cycle transposes and 256 one-row rowsum matmuls.
"""

import sys

sys.path.insert(0, "/opt/trn_rl_repo")

import numpy as np
import ml_dtypes
from ml_dtypes import bfloat16

E4NP = ml_dtypes.float8_e4m3

import concourse.bass as bass
import concourse.mybir as mybir
import concourse.tile as tile
from concourse import bacc

B, N, E = 4, 2048, 1024
NQ = N // 2          # query rows per core
P = 128              # partitions
FP = 4               # e-tile pairs (contraction E = 4 * 256)
MT = 16              # key tiles
MP = 8               # key tile pairs
QT = 8               # query blocks of 128
H = NQ // 2          # 512-column matmul halves (one PSUM bank)
F32 = mybir.dt.float32
BF16 = mybir.dt.bfloat16
F8 = mybir.dt.float8e4
DR = mybir.MatmulPerfMode.DoubleRow
EXP_SCALE = 1.0 / 128.0
R_SCALE = 32.0
RECF_SCALE = 16.0    # W28 pre-scale folded out of the final reciprocal


def build_program():
    nc = bacc.Bacc("TRN2", target_bir_lowering=False, debug=False)
    a8p = nc.dram_tensor("a8p", [FP, P, 2 * 2 * E], F8,
                         kind="ExternalInput").ap()
    xq8p = nc.dram_tensor("xq8p", [FP, P, 2 * 2 * NQ], F8,
                          kind="ExternalInput").ap()
    xk8p = nc.dram_tensor("xk8p", [FP, P, 2 * 2 * NQ], F8,
                          kind="ExternalInput").ap()
    xn8p = nc.dram_tensor("xn8p", [MP, P, 2 * 2 * E], F8,
                          kind="ExternalInput").ap()
    w28p = nc.dram_tensor("w28p", [FP, P, 2 * 2 * E], F8,
                          kind="ExternalInput").ap()
    bout = nc.dram_tensor("bout", [E], BF16, kind="ExternalInput").ap()
    ident = nc.dram_tensor("ident", [P, P], BF16, kind="ExternalInput").ap()
    rsc = nc.dram_tensor("rsc", [2, 4, P], BF16, kind="Internal").ap()
    y = nc.dram_tensor("y", [NQ, E], BF16, kind="ExternalOutput").ap()

    with tile.TileContext(nc) as tc:
        _body(nc, tc, a8p, xq8p, xk8p, xn8p, w28p, bout, ident, rsc, y)
    nc.compile()
    return nc


def _body(nc, tc, a8p, xq8p, xk8p, xn8p, w28p, bout, ident, rsc, y):
    Exp = mybir.ActivationFunctionType.Exp
    Copy = mybir.ActivationFunctionType.Copy
    Add = mybir.AluOpType.add
    Sub = mybir.AluOpType.subtract
    Mul = mybir.AluOpType.mult

    cst = tc.alloc_tile_pool(name="cst", bufs=1)
    warm = cst.tile([P, 256], BF16, name="warm", tag="warm")
    nc.vector.memset(warm, 0.0)
    ones = cst.tile([P, 1], BF16, name="ones", tag="ones")
    nc.vector.memset(ones, 1.0)
    ones8 = cst.tile([P, 2, 1], F8, name="ones8", tag="ones8")
    nc.vector.memset(ones8, 1.0)

    # ---- input loads; issue order tracks first-use order ----
    t8p = tc.alloc_tile_pool(name="t8p", bufs=1)
    T8h_t = [t8p.tile([P, 2, NQ], F8, name=f"T8h{f}", tag=f"T8h{f}")
             for f in range(FP)]
    T8l_t = [t8p.tile([P, 2, NQ], F8, name=f"T8l{f}", tag=f"T8l{f}")
             for f in range(FP)]

    pa = tc.alloc_tile_pool(name="pa", bufs=1)

    def hl_views(tile_, cols):
        hi = tile_[:, 0:2 * cols].rearrange("p (i c) -> p i c", i=2)
        lo = tile_[:, 2 * cols:4 * cols].rearrange("p (i c) -> p i c", i=2)
        return hi, lo

    def quad_views(tile_):
        """[P, 4096] pack -> 4 views [P, 2, 512]: hiA, loA, hiB, loB."""
        return [tile_[:, k * 1024:(k + 1) * 1024]
                .rearrange("p (i c) -> p i c", i=2) for k in range(4)]

    a8q, xqq, xkq = [], [], []
    ap_t, xqp_t, xkp_t = [], [], []
    for f in range(FP):
        at = pa.tile([P, 4 * E], F8, name=f"a8p{f}", tag=f"a8p{f}")
        qt_ = cst.tile([P, 4 * NQ], F8, name=f"xq8p{f}", tag=f"xq8p{f}")
        kt = cst.tile([P, 4 * NQ], F8, name=f"xk8p{f}", tag=f"xk8p{f}")
        ap_t.append(at); xqp_t.append(qt_); xkp_t.append(kt)
        a8q.append(quad_views(at))
        xqq.append(quad_views(qt_))
        xkq.append(quad_views(kt))
    # Group-1 chunks (fo 0-3 weights + h0 columns), consumption order.
    # fp0 rides the scalar HWDGE, split hi-first, so chains start sooner.
    nc.scalar.dma_start(out=ap_t[0][:, 0:E], in_=a8p[0][:, 0:E])
    nc.scalar.dma_start(out=xqp_t[0][:, 0:NQ], in_=xq8p[0][:, 0:NQ])
    nc.scalar.dma_start(out=ap_t[0][:, E:2 * E], in_=a8p[0][:, E:2 * E])
    nc.scalar.dma_start(out=xqp_t[0][:, NQ:2 * NQ], in_=xq8p[0][:, NQ:2 * NQ])
    for f in range(1, FP):
        nc.sync.dma_start(out=ap_t[f][:, 0:2 * E], in_=a8p[f][:, 0:2 * E])
        nc.sync.dma_start(out=xqp_t[f][:, 0:2 * NQ], in_=xq8p[f][:, 0:2 * NQ])
    ident_t = cst.tile([P, P], BF16, name="ident_t", tag="ident_t")
    nc.scalar.dma_start(out=ident_t, in_=ident)
    bo_b = cst.tile([P, E], BF16, name="bo_b", tag="bo_b")
    bout_bcast = bass.AP(tensor=bout.tensor, offset=0, ap=[[0, P], [1, E]])
    nc.scalar.dma_start(out=bo_b, in_=bout_bcast)
    # Group-2 (h1 columns), then group-3/4 (fo 4-7 weights)
    for f in range(FP):
        nc.sync.dma_start(out=xqp_t[f][:, 2 * NQ:], in_=xq8p[f][:, 2 * NQ:])
    for f in range(FP):
        nc.sync.dma_start(out=ap_t[f][:, 2 * E:], in_=a8p[f][:, 2 * E:])
    # xk8: S needs chunk A from mt>=8, chunk B from mt>=12
    for f in range(FP):
        nc.sync.dma_start(out=xkp_t[f][:, 0:2 * NQ], in_=xk8p[f][:, 0:2 * NQ])
    for f in range(FP):
        nc.sync.dma_start(out=xkp_t[f][:, 2 * NQ:], in_=xk8p[f][:, 2 * NQ:])
    xn8h_t, xn8l_t = [], []
    for m in range(MP):
        nt = cst.tile([P, 4 * E], F8, name=f"xn8p{m}", tag=f"xn8p{m}")
        nc.sync.dma_start(out=nt, in_=xn8p[m])
        hi, lo = hl_views(nt, E)
        xn8h_t.append(hi); xn8l_t.append(lo)
    w28h_t, w28l_t = [], []
    for f in range(FP):
        wt = cst.tile([P, 4 * E], F8, name=f"w28p{f}", tag=f"w28p{f}")
        nc.sync.dma_start(out=wt, in_=w28p[f])
        hi, lo = hl_views(wt, E)
        w28h_t.append(hi); w28l_t.append(lo)

    # ---- PE pstate warmup while first loads land ----
    with tc.tile_pool(name="wps", bufs=1, space="PSUM") as wpp:
        wps = wpp.tile([P, 256], F32, name="wps", tag="wps")
        for i in range(7):
            nc.tensor.matmul(wps, warm[:, 0:P], warm, start=True, stop=True)

    def terms(wh, wl, rh, rl):
        return ((wh, rh), (wl, rh), (wh, rl))

    # ---- S, softmax pipeline, U, Y ----
    pres = tc.alloc_tile_pool(name="pres", bufs=1, side="right")
    p_t = [pres.tile([P, NQ], BF16, name=f"p{m}", tag=f"p{m}")
           for m in range(MT)]
    p8p = tc.alloc_tile_pool(name="p8p", bufs=1, side="right")
    p8h_t = [p8p.tile([P, 2, NQ], F8, name=f"p8h{m}", tag=f"p8h{m}")
             for m in range(MP)]
    p8l_t = [p8p.tile([P, 2, NQ], F8, name=f"p8l{m}", tag=f"p8l{m}")
             for m in range(MP)]
    u8p = tc.alloc_tile_pool(name="u8p", bufs=1, side="right")
    U8h_t = [u8p.tile([P, 2, NQ], F8, name=f"U8h{f}", tag=f"U8h{f}")
             for f in range(FP)]
    U8l_t = [u8p.tile([P, 2, NQ], F8, name=f"U8l{f}", tag=f"U8l{f}")
             for f in range(FP)]
    smp = tc.alloc_tile_pool(name="smp", bufs=1, side="right")
    sums_acc = [smp.tile([P, 4], F32, name=f"sums_acc{h}", tag=f"sums_acc{h}")
                for h in range(2)]
    r8_acc = [smp.tile([P, 4], F32, name=f"r8_acc{h}", tag=f"r8_acc{h}")
              for h in range(2)]
    recf = [smp.tile([P, 4], F32, name=f"recf{h}", tag=f"recf{h}")
            for h in range(2)]
    rrow = [smp.tile([4, P], BF16, name=f"rrow{h}", tag=f"rrow{h}")
            for h in range(2)]
    r_rep = [smp.tile([P, H], BF16, name=f"r_rep{h}", tag=f"r_rep{h}")
             for h in range(2)]
    tmpp = tc.alloc_tile_pool(name="tmpp", bufs=4, side="right")

    def s_chain(h, mt, spp):
        xsq = xqq if mt < 8 else xkq
        mm = mt % 8
        hk, c = mm // 4, mm % 4
        s = spp.tile([P, H], F32, name="s", tag="chain")
        for fp in range(FP):
            xh, xl = xsq[fp][2 * hk], xsq[fp][2 * hk + 1]
            for t, (wa, rb) in enumerate(
                    terms(xh, xl, T8h_t[fp], T8l_t[fp])):
                nc.tensor.matmul(
                    s, wa[:, :, c * P:(c + 1) * P],
                    rb[:, :, h * H:(h + 1) * H],
                    start=(fp == 0 and t == 0),
                    stop=(fp == FP - 1 and t == 2), perf_mode=DR)
        nc.scalar.activation(p_t[mt][:, h * H:(h + 1) * H], s, Exp,
                             scale=EXP_SCALE)

    def row_sums(h, mt, sumsp):
        sm = sumsp.tile([P, 4], F32, name="sums_m", tag="sums_m")
        for q in range(4):
            nc.tensor.matmul(
                sm[:, q:q + 1],
                p_t[mt][:, h * H + q * P: h * H + (q + 1) * P], ones,
                start=True, stop=True)
        if mt == 0:
            nc.vector.tensor_copy(sums_acc[h], sm)
        else:
            nc.vector.tensor_tensor(out=sums_acc[h], in0=sums_acc[h],
                                    in1=sm, op=Add)

    def r_path(h, rtpp):
        """sums_acc[h] -> r_rep[h] broadcast row (32/rowsum as bf16)."""
        rf = tmpp.tile([P, 4], F32, name="rf", tag="rf")
        nc.vector.reciprocal(rf, sums_acc[h])
        rb16 = tmpp.tile([P, 4], BF16, name="rb16", tag="rb16")
        nc.vector.tensor_scalar_mul(rb16, rf, R_SCALE)
        rtp = rtpp.tile([4, P], BF16, name="rtp", tag="rtp")
        nc.tensor.transpose(rtp, rb16, ident_t)
        nc.vector.tensor_copy(rrow[h], rtp)
        nc.gpsimd.dma_start(out=rsc[h], in_=rrow[h])
        rsc_b = bass.AP(tensor=rsc.tensor, offset=h * 4 * P,
                        ap=[[0, P], [1, H]])
        nc.gpsimd.dma_start(out=r_rep[h], in_=rsc_b)

    def p8_prod(h, mt, lo_eng=None):
        mp, i = mt // 2, mt % 2
        tmp = tmpp.tile([P, H], BF16, name="tmp", tag="tmp")
        nc.vector.tensor_tensor(
            out=tmp, in0=p_t[mt][:, h * H:(h + 1) * H],
            in1=r_rep[h], op=Mul)
        hs = p8h_t[mp][:, i, h * H:(h + 1) * H]
        nc.scalar.activation(hs, tmp, Copy)
        (lo_eng or nc.vector).tensor_tensor(
            out=p8l_t[mp][:, i, h * H:(h + 1) * H],
            in0=tmp, in1=hs, op=Sub)

    def u_chain(hh, fo, upp):
        u = upp.tile([P, H], F32, name="u", tag="chain")
        for mp in range(MP):
            for t, (wa, rb) in enumerate(
                    terms(xn8h_t[mp], xn8l_t[mp], p8h_t[mp], p8l_t[mp])):
                nc.tensor.matmul(
                    u, wa[:, :, fo * P:(fo + 1) * P],
                    rb[:, :, hh * H:(hh + 1) * H],
                    start=(mp == 0 and t == 0),
                    stop=(mp == MP - 1 and t == 2), perf_mode=DR)
        hs = U8h_t[fo // 2][:, fo % 2, hh * H:(hh + 1) * H]
        nc.scalar.activation(hs, u, Copy)
        nc.vector.tensor_tensor(
            out=U8l_t[fo // 2][:, fo % 2, hh * H:(hh + 1) * H],
            in0=u, in1=hs, op=Sub)

    def r8_sums(hh, mp, sumsp):
        """rowsums of p8 (hi+lo) for query blocks of half hh."""
        for j, px in enumerate((p8h_t, p8l_t)):
            sm = sumsp.tile([P, 4], F32, name="r8_m", tag="sums_m")
            for q in range(4):
                nc.tensor.matmul(
                    sm[:, q:q + 1],
                    px[mp][:, :, hh * H + q * P: hh * H + (q + 1) * P],
                    ones8, start=True, stop=True, perf_mode=DR)
            if mp == 0 and j == 0:
                nc.vector.tensor_copy(r8_acc[hh], sm)
            else:
                nc.vector.tensor_tensor(out=r8_acc[hh], in0=r8_acc[hh],
                                        in1=sm, op=Add)

    def recf_path(hh):
        r8s = tmpp.tile([P, 4], F32, name="r8s", tag="rf")
        nc.vector.tensor_scalar_mul(r8s, r8_acc[hh], RECF_SCALE)
        nc.vector.reciprocal(recf[hh], r8s)

    def y_chain(qt, he, ypp, ysp):
        hh = qt // 4
        yps = ypp.tile([P, H], F32, name="yps", tag="chain")
        for fp in range(FP):
            for t, (wa, rb) in enumerate(
                    terms(U8h_t[fp], U8l_t[fp], w28h_t[fp], w28l_t[fp])):
                nc.tensor.matmul(
                    yps, wa[:, :, qt * P:(qt + 1) * P],
                    rb[:, :, he * H:(he + 1) * H],
                    start=(fp == 0 and t == 0),
                    stop=(fp == FP - 1 and t == 2), perf_mode=DR)
        ysb = ysp.tile([P, H], BF16, name="ysb", tag="ysb")
        npieces = 2 if (qt, he) in ((6, 1), (7, 0), (7, 1)) else 1
        w = H // npieces
        for pc in range(npieces):
            sl = slice(pc * w, (pc + 1) * w)
            nc.scalar.activation(ysb[:, sl], yps[:, sl], Copy,
                                 scale=recf[hh][:, (qt % 4):(qt % 4) + 1])
            nc.vector.tensor_tensor(out=ysb[:, sl], in0=ysb[:, sl],
                                    in1=bo_b[:, he * H + pc * w:
                                             he * H + (pc + 1) * w], op=Add)
            if qt >= 6:
                eng = (nc.sync, nc.gpsimd)[(2 * he + pc) % 2]
            else:
                eng = (nc.gpsimd, nc.sync, nc.scalar)[(2 * he + pc + qt) % 3]
            eng.dma_start(
                out=y[qt * P:(qt + 1) * P,
                      he * H + pc * w: he * H + (pc + 1) * w],
                in_=ysb[:, sl])

    with tc.tile_pool(name="smps", bufs=2, space="PSUM") as sumsp, \
         tc.tile_pool(name="rtps", bufs=1, space="PSUM") as rtpp, \
         tc.tile_pool(name="workp", bufs=5, space="PSUM") as workp, \
         tc.tile_pool(name="ysb_p", bufs=6, side="right") as ysp:
        # ---- T: 4 groups of 4 chains, fp-outer interleaved per group ----
        for fos, h in ((range(0, 4), 0), (range(0, 4), 1),
                       (range(4, 8), 0), (range(4, 8), 1)):
            tps = {fo: workp.tile([P, H], F32, name=f"t{fo}_{h}", tag="chain")
                   for fo in fos}
            g = fos[0] // 4
            for fp in range(FP):
                ah, al = a8q[fp][2 * g], a8q[fp][2 * g + 1]
                qh, ql = xqq[fp][2 * h], xqq[fp][2 * h + 1]
                for t, (wa, rb) in enumerate(terms(ah, al, qh, ql)):
                    for fo in fos:
                        c = fo % 4
                        nc.tensor.matmul(
                            tps[fo], wa[:, :, c * P:(c + 1) * P], rb,
                            start=(fp == 0 and t == 0),
                            stop=(fp == FP - 1 and t == 2), perf_mode=DR)
            for fo in fos:
                hs = T8h_t[fo // 2][:, fo % 2, h * H:(h + 1) * H]
                nc.scalar.activation(hs, tps[fo], Copy)
                nc.vector.tensor_tensor(
                    out=T8l_t[fo // 2][:, fo % 2, h * H:(h + 1) * H],
                    in0=tps[fo], in1=hs, op=Sub)
        pa.release()

        # ---- S half 0 ----
        for mt in range(MT):
            s_chain(0, mt, workp)
            row_sums(0, mt, sumsp)
        # ---- S half 1, h0 softmax pipeline interleaved ----
        s_chain(1, 0, workp)
        row_sums(1, 0, sumsp)
        r_path(0, rtpp)
        for mt in range(1, MT):
            s_chain(1, mt, workp)
            row_sums(1, mt, sumsp)
            p8_prod(0, mt - 1)
        p8_prod(0, MT - 1)
        # ---- U half 0; h1 softmax pipeline + h0 fp8 rowsums ----
        u_chain(0, 0, workp)
        r_path(1, rtpp)
        for fo in range(1, 8):
            u_chain(0, fo, workp)
            p8_prod(1, 2 * (fo - 1), lo_eng=nc.vector)
            p8_prod(1, 2 * (fo - 1) + 1, lo_eng=nc.gpsimd)
            r8_sums(0, fo - 1, sumsp)
        p8_prod(1, MT - 2, lo_eng=nc.vector)
        p8_prod(1, MT - 1, lo_eng=nc.gpsimd)
        r8_sums(0, MP - 1, sumsp)
        recf_path(0)
        # ---- U half 1; h1 fp8 rowsums ----
        for fo in range(8):
            u_chain(1, fo, workp)
            if fo < MP:
                r8_sums(1, fo, sumsp)
        recf_path(1)
        # ---- Y ----
        for qt in (0, 1, 2, 3):
            for he in range(2):
                y_chain(qt, he, workp, ysp)
        for qt in (4, 5, 6, 7):
            for he in range(2):
                y_chain(qt, he, workp, ysp)

    tmpp.release()
    smp.release()
    u8p.release()
    p8p.release()
    pres.release()
    t8p.release()
    cst.release()


_NC_CACHE = None


def _get_program():
    global _NC_CACHE
    if _NC_CACHE is None:
        _NC_CACHE = build_program()
    return _NC_CACHE


def _split8(v):
    hi = v.astype(E4NP)
    lo = (v - hi.astype(np.float32)).astype(E4NP)
    return hi, lo


def _pack_fp(mat, cols):
    """[E, cols] -> [FP, P, 2, cols] pair-packed along the contraction dim."""
    return np.ascontiguousarray(
        mat.reshape(FP, 2, P, cols).transpose(0, 2, 1, 3))


def _pack_hl(hi, lo, nt, cols):
    """pair-packed hi/lo [nt, P, 2, cols] -> [nt, P, 4*cols] row-concat."""
    out = np.empty((nt, P, 4 * cols), dtype=hi.dtype)
    out[:, :, 0:2 * cols] = hi.reshape(nt, P, 2 * cols)
    out[:, :, 2 * cols:] = lo.reshape(nt, P, 2 * cols)
    return out


def _pack_quad(hi, lo, nt, cols):
    """[nt, P, 2, cols] hi/lo -> [nt, P, 4*cols] as hiA|loA|hiB|loB,
    where A = cols[0:cols//2], B = cols[cols//2:]."""
    hw = cols // 2
    out = np.empty((nt, P, 4 * cols), dtype=hi.dtype)
    out[:, :, 0 * cols:1 * cols] = hi[:, :, :, 0:hw].reshape(nt, P, cols)
    out[:, :, 1 * cols:2 * cols] = lo[:, :, :, 0:hw].reshape(nt, P, cols)
    out[:, :, 2 * cols:3 * cols] = hi[:, :, :, hw:].reshape(nt, P, cols)
    out[:, :, 3 * cols:4 * cols] = lo[:, :, :, hw:].reshape(nt, P, cols)
    return out


def _host_prep(x, W_qkv, W_out, b_out):
    Wq = W_qkv[:, :E].astype(np.float64)
    Wk = W_qkv[:, E:2 * E].astype(np.float64)
    Wv = W_qkv[:, 2 * E:].astype(np.float64)
    A = ((Wq @ Wk.T) * (0.125 * 128.0)).astype(np.float32)
    W2 = ((Wv @ W_out.astype(np.float64)) * 16.0).astype(np.float32)
    A8h, A8l = _split8(A)
    W28h, W28l = _split8(W2)
    a8_p = _pack_quad(_pack_fp(A8h, E), _pack_fp(A8l, E), FP, E)
    w28_p = _pack_hl(_pack_fp(W28h, E), _pack_fp(W28l, E), FP, E)
    bo = b_out.astype(bfloat16)
    identity = np.eye(P, dtype=bfloat16)

    in_maps = []
    for c in range(8):
        b, half = divmod(c, 2)
        xb = np.asarray(x[b], dtype=np.float32)
        xrot = np.concatenate([xb[half * NQ:], xb[:half * NQ]], axis=0)
        x8h, x8l = _split8(xrot)
        xt8h = _pack_fp(np.ascontiguousarray(x8h.T), N)
        xt8l = _pack_fp(np.ascontiguousarray(x8l.T), N)
        xn8h = np.ascontiguousarray(
            x8h.reshape(MP, 2, P, E).transpose(0, 2, 1, 3))
        xn8l = np.ascontiguousarray(
            x8l.reshape(MP, 2, P, E).transpose(0, 2, 1, 3))
        in_maps.append({
            "a8p": a8_p,
            "xq8p": _pack_quad(xt8h[:, :, :, 0:NQ], xt8l[:, :, :, 0:NQ],
                               FP, NQ),
            "xk8p": _pack_quad(xt8h[:, :, :, NQ:N], xt8l[:, :, :, NQ:N],
                               FP, NQ),
            "xn8p": _pack_hl(xn8h, xn8l, MP, E),
            "w28p": w28_p,
            "bout": bo, "ident": identity,
        })
    return in_maps


def kernel(x, W_qkv, W_out, b_out):
    from concourse.bass_utils import run_bass_kernel_spmd

    x = np.asarray(x, dtype=np.float32)
    W_qkv = np.asarray(W_qkv, dtype=np.float32)
    W_out = np.asarray(W_out, dtype=np.float32)
    b_out = np.asarray(b_out, dtype=np.float32)

    nc = _get_program()
    in_maps = _host_prep(x, W_qkv, W_out, b_out)
    res = run_bass_kernel_spmd(nc, in_maps, list(range(8)))
    out = np.empty((B, N, E), dtype=np.float32)
    for c in range(8):
        b, half = divmod(c, 2)
        out[b, half * NQ:(half + 1) * NQ] = res.results[c]["y"].astype(
            np.float32)
    return out


# revision 31
# speedup vs baseline: 1.2426x; 1.0089x over previous
"""Classical self-attention (head-summed scores) on 8 trn2 NeuronCores.

Algebraic rewrite (as baseline): scores sum over heads AND head dim, so
  S = x A x^T with A = Wq Wk^T / 8,  out = softmax(S) x W2 + b, W2 = Wv Wout.
A and W2 are folded on the host.

v2: every matmul phase runs as fp8e4m3 DoubleRow (perf_mode) with hi/lo
splitting.  Each operand v is stored as v_hi = e4m3(v), v_lo = e4m3(v - v_hi);
a product contracts 256 elements per instruction (pair-packed along the
contraction dim) via three term-matmuls (hi*hi, hi*lo, lo*hi), costing
1.5 x 0.5 = 0.75 cycles/row-of-128 vs bf16's 1.0 -> ~123us PE floor
(294912 rows) instead of 164us.  Measured end-to-end rel err ~5e-3
(better than the bf16 baseline's 8.4e-3: the e4m3 hi+lo pair carries ~9
mantissa bits).

Ranges (e4m3 max normal is 240, overflow -> inf, so everything is scaled):
  A8 = 128*A   (std ~0.5), x8 = x (std 1), T_psum = 128*T (std 16)
  T8 = split8(T_psum) directly from PSUM, exp scale 1/128
  P = exp(S/1) bf16 unscaled; r = 32/rowsum(P) per query -> p8 = split8(P*r)
  (attention weights <= 1 so P*r <= 32; per-query scale cancels because the
  final normalizer is recomputed from p8 itself)
  U_psum = U*r (<=176), U8 = split8(U_psum); W28 = split8(16*W2)
  out = Y_psum / (16 * rowsum(p8)) + b

Schedule: S and U are split by query halves h in {0,1} (512 cols each).
While PE runs S(h=1), the h=0 pipeline (rowsums -> r -> PE-transpose ->
DRAM bounce -> broadcast row -> p8 tiles) runs on ACT/DVE/DMA; U(hh=0)
consumes finished p8 h0 tiles while the h1 pipeline fills, and Y(qt 0-3)
overlaps U(hh=1).  The only PE instructions off the main chain are two
128-cycle transposes and 256 one-row rowsum matmuls.
"""

import sys

sys.path.insert(0, "/opt/trn_rl_repo")

import numpy as np
import ml_dtypes
from ml_dtypes import bfloat16

E4NP = ml_dtypes.float8_e4m3

import concourse.bass as bass
import concourse.mybir as mybir
import concourse.tile as tile
from concourse import bacc

B, N, E = 4, 2048, 1024
NQ = N // 2          # query rows per core
P = 128              # partitions
FP = 4               # e-tile pairs (contraction E = 4 * 256)
MT = 16              # key tiles
MP = 8               # key tile pairs
QT = 8               # query blocks of 128
H = NQ // 2          # 512-column matmul halves (one PSUM bank)
F32 = mybir.dt.float32
BF16 = mybir.dt.bfloat16
F8 = mybir.dt.float8e4
DR = mybir.MatmulPerfMode.DoubleRow
EXP_SCALE = 1.0 / 128.0
R_SCALE = 32.0
RECF_SCALE = 16.0    # W28 pre-scale folded out of the final reciprocal


def build_program():
    nc = bacc.Bacc("TRN2", target_bir_lowering=False, debug=False)
    a8p = nc.dram_tensor("a8p", [FP, P, 2 * 2 * E], F8,
                         kind="ExternalInput").ap()
    xq8p = nc.dram_tensor("xq8p", [FP, P, 2 * 2 * NQ], F8,
                          kind="ExternalInput").ap()
    xk8p = nc.dram_tensor("xk8p", [FP, P, 2 * 2 * NQ], F8,
                          kind="ExternalInput").ap()
    xn8p = nc.dram_tensor("xn8p", [MP, P, 2 * 2 * E], F8,
                          kind="ExternalInput").ap()
    w28p = nc.dram_tensor("w28p", [FP, P, 2 * 2 * E], F8,
                          kind="ExternalInput").ap()
    bout = nc.dram_tensor("bout", [E], BF16, kind="ExternalInput").ap()
    ident = nc.dram_tensor("ident", [P, P], BF16, kind="ExternalInput").ap()
    rsc = nc.dram_tensor("rsc", [2, 4, P], BF16, kind="Internal").ap()
    y = nc.dram_tensor("y", [NQ, E], BF16, kind="ExternalOutput").ap()

    with tile.TileContext(nc) as tc:
        _body(nc, tc, a8p, xq8p, xk8p, xn8p, w28p, bout, ident, rsc, y)
    nc.compile()
    return nc


def _body(nc, tc, a8p, xq8p, xk8p, xn8p, w28p, bout, ident, rsc, y):
    Exp = mybir.ActivationFunctionType.Exp
    Copy = mybir.ActivationFunctionType.Copy
    Add = mybir.AluOpType.add
    Sub = mybir.AluOpType.subtract
    Mul = mybir.AluOpType.mult

    cst = tc.alloc_tile_pool(name="cst", bufs=1)
    warm = cst.tile([P, 256], BF16, name="warm", tag="warm")
    nc.gpsimd.memset(warm, 0.0)
    ones = cst.tile([P, 1], BF16, name="ones", tag="ones")
    nc.vector.memset(ones, 1.0)
    ones8 = cst.tile([P, 2, 1], F8, name="ones8", tag="ones8")
    nc.vector.memset(ones8, 1.0)

    # ---- input loads; issue order tracks first-use order ----
    t8p = tc.alloc_tile_pool(name="t8p", bufs=1)
    T8h_t = [t8p.tile([P, 2, NQ], F8, name=f"T8h{f}", tag=f"T8h{f}")
             for f in range(FP)]
    T8l_t = [t8p.tile([P, 2, NQ], F8, name=f"T8l{f}", tag=f"T8l{f}")
             for f in range(FP)]

    pa = tc.alloc_tile_pool(name="pa", bufs=1)

    def hl_views(tile_, cols):
        hi = tile_[:, 0:2 * cols].rearrange("p (i c) -> p i c", i=2)
        lo = tile_[:, 2 * cols:4 * cols].rearrange("p (i c) -> p i c", i=2)
        return hi, lo

    def quad_views(tile_):
        """[P, 4096] pack -> 4 views [P, 2, 512]: hiA, loA, hiB, loB."""
        return [tile_[:, k * 1024:(k + 1) * 1024]
                .rearrange("p (i c) -> p i c", i=2) for k in range(4)]

    a8q, xqq, xkq = [], [], []
    ap_t, xqp_t, xkp_t = [], [], []
    for f in range(FP):
        at = pa.tile([P, 4 * E], F8, name=f"a8p{f}", tag=f"a8p{f}")
        qt_ = cst.tile([P, 4 * NQ], F8, name=f"xq8p{f}", tag=f"xq8p{f}")
        kt = cst.tile([P, 4 * NQ], F8, name=f"xk8p{f}", tag=f"xk8p{f}")
        ap_t.append(at); xqp_t.append(qt_); xkp_t.append(kt)
        a8q.append(quad_views(at))
        xqq.append(quad_views(qt_))
        xkq.append(quad_views(kt))
    # Group-1 chunks (fo 0-3 weights + h0 columns), consumption order.
    # fp0 rides the scalar HWDGE, split hi-first, so chains start sooner.
    nc.scalar.dma_start(out=ap_t[0][:, 0:E], in_=a8p[0][:, 0:E])
    nc.scalar.dma_start(out=xqp_t[0][:, 0:NQ], in_=xq8p[0][:, 0:NQ])
    nc.scalar.dma_start(out=ap_t[0][:, E:2 * E], in_=a8p[0][:, E:2 * E])
    nc.scalar.dma_start(out=xqp_t[0][:, NQ:2 * NQ], in_=xq8p[0][:, NQ:2 * NQ])
    for f in range(1, FP):
        nc.sync.dma_start(out=ap_t[f][:, 0:2 * E], in_=a8p[f][:, 0:2 * E])
        nc.sync.dma_start(out=xqp_t[f][:, 0:2 * NQ], in_=xq8p[f][:, 0:2 * NQ])
    ident_t = cst.tile([P, P], BF16, name="ident_t", tag="ident_t")
    nc.scalar.dma_start(out=ident_t, in_=ident)
    bo_b = cst.tile([P, E], BF16, name="bo_b", tag="bo_b")
    bout_bcast = bass.AP(tensor=bout.tensor, offset=0, ap=[[0, P], [1, E]])
    nc.scalar.dma_start(out=bo_b, in_=bout_bcast)
    # Group-2 (h1 columns), then group-3/4 (fo 4-7 weights)
    for f in range(FP):
        nc.sync.dma_start(out=xqp_t[f][:, 2 * NQ:], in_=xq8p[f][:, 2 * NQ:])
    for f in range(FP):
        nc.sync.dma_start(out=ap_t[f][:, 2 * E:], in_=a8p[f][:, 2 * E:])
    # xk8: S needs chunk A from mt>=8, chunk B from mt>=12
    for f in range(FP):
        nc.sync.dma_start(out=xkp_t[f][:, 0:2 * NQ], in_=xk8p[f][:, 0:2 * NQ])
    for f in range(FP):
        nc.sync.dma_start(out=xkp_t[f][:, 2 * NQ:], in_=xk8p[f][:, 2 * NQ:])
    xn8h_t, xn8l_t = [], []
    for m in range(MP):
        nt = cst.tile([P, 4 * E], F8, name=f"xn8p{m}", tag=f"xn8p{m}")
        nc.sync.dma_start(out=nt, in_=xn8p[m])
        hi, lo = hl_views(nt, E)
        xn8h_t.append(hi); xn8l_t.append(lo)
    w28h_t, w28l_t = [], []
    for f in range(FP):
        wt = cst.tile([P, 4 * E], F8, name=f"w28p{f}", tag=f"w28p{f}")
        nc.sync.dma_start(out=wt, in_=w28p[f])
        hi, lo = hl_views(wt, E)
        w28h_t.append(hi); w28l_t.append(lo)

    def terms(wh, wl, rh, rl):
        return ((wh, rh), (wl, rh), (wh, rl))

    # ---- S, softmax pipeline, U, Y ----
    pres = tc.alloc_tile_pool(name="pres", bufs=1, side="right")
    p_t = [pres.tile([P, 2, NQ], BF16, name=f"p{m}", tag=f"p{m}")
           for m in range(MP)]
    p8p = tc.alloc_tile_pool(name="p8p", bufs=1, side="right")
    p8h_t = [p8p.tile([P, 2, NQ], F8, name=f"p8h{m}", tag=f"p8h{m}")
             for m in range(MP)]
    p8l_t = [p8p.tile([P, 2, NQ], F8, name=f"p8l{m}", tag=f"p8l{m}")
             for m in range(MP)]
    u8p = tc.alloc_tile_pool(name="u8p", bufs=1, side="right")
    U8h_t = [u8p.tile([P, 2, NQ], F8, name=f"U8h{f}", tag=f"U8h{f}")
             for f in range(FP)]
    U8l_t = [u8p.tile([P, 2, NQ], F8, name=f"U8l{f}", tag=f"U8l{f}")
             for f in range(FP)]
    smp = tc.alloc_tile_pool(name="smp", bufs=1, side="right")
    sums_acc = [smp.tile([P, 4], F32, name=f"sums_acc{h}", tag=f"sums_acc{h}")
                for h in range(2)]
    r8_acc = [smp.tile([P, 4], F32, name=f"r8_acc{h}", tag=f"r8_acc{h}")
              for h in range(2)]
    recf = [smp.tile([P, 4], F32, name=f"recf{h}", tag=f"recf{h}")
            for h in range(2)]
    rrow = [smp.tile([4, P], BF16, name=f"rrow{h}", tag=f"rrow{h}")
            for h in range(2)]
    r_rep = [smp.tile([P, 2, H], BF16, name=f"r_rep{h}", tag=f"r_rep{h}")
             for h in range(2)]
    tmpp = tc.alloc_tile_pool(name="tmpp", bufs=4, side="right")

    def s_chain(h, mt, spp):
        xsq = xqq if mt < 8 else xkq
        mm = mt % 8
        hk, c = mm // 4, mm % 4
        s = spp.tile([P, H], F32, name="s", tag="chain")
        for fp in range(FP):
            xh, xl = xsq[fp][2 * hk], xsq[fp][2 * hk + 1]
            for t, (wa, rb) in enumerate(
                    terms(xh, xl, T8h_t[fp], T8l_t[fp])):
                nc.tensor.matmul(
                    s, wa[:, :, c * P:(c + 1) * P],
                    rb[:, :, h * H:(h + 1) * H],
                    start=(fp == 0 and t == 0),
                    stop=(fp == FP - 1 and t == 2), perf_mode=DR)
        nc.scalar.activation(p_t[mt // 2][:, mt % 2, h * H:(h + 1) * H],
                             s, Exp, scale=EXP_SCALE)

    def row_sums(h, mt, sumsp):
        sm = sumsp.tile([P, 4], F32, name="sums_m", tag="sums_m")
        for q in range(4):
            nc.tensor.matmul(
                sm[:, q:q + 1],
                p_t[mt // 2][:, mt % 2,
                             h * H + q * P: h * H + (q + 1) * P], ones,
                start=True, stop=True)
        if mt == 0:
            nc.vector.tensor_copy(sums_acc[h], sm)
        else:
            nc.vector.tensor_tensor(out=sums_acc[h], in0=sums_acc[h],
                                    in1=sm, op=Add)

    def r_path(h, rtpp):
        """sums_acc[h] -> r_rep[h] broadcast row (32/rowsum as bf16)."""
        rf = tmpp.tile([P, 4], F32, name="rf", tag="rf")
        nc.vector.reciprocal(rf, sums_acc[h])
        rb16 = tmpp.tile([P, 4], BF16, name="rb16", tag="rb16")
        nc.vector.tensor_scalar_mul(rb16, rf, R_SCALE)
        rtp = rtpp.tile([4, P], BF16, name="rtp", tag="sums_m")
        nc.tensor.transpose(rtp, rb16, ident_t)
        nc.vector.tensor_copy(rrow[h], rtp)
        nc.sync.dma_start(out=rsc[h], in_=rrow[h])
        rsc_b = bass.AP(tensor=rsc.tensor, offset=h * 4 * P,
                        ap=[[0, P], [0, 2], [1, H]])
        nc.sync.dma_start(out=r_rep[h], in_=rsc_b)

    def p8_prod(h, mp, lo_eng=None):
        tmp = tmpp.tile([P, 2, H], BF16, name="tmp", tag="tmp")
        nc.vector.tensor_tensor(
            out=tmp, in0=p_t[mp][:, :, h * H:(h + 1) * H],
            in1=r_rep[h], op=Mul)
        hs = p8h_t[mp][:, :, h * H:(h + 1) * H]
        nc.scalar.activation(hs, tmp, Copy)
        (lo_eng or nc.vector).tensor_tensor(
            out=p8l_t[mp][:, :, h * H:(h + 1) * H],
            in0=tmp, in1=hs, op=Sub)

    def u_chain(hh, fo, upp, lo_eng=None):
        u = upp.tile([P, H], F32, name="u", tag="chain")
        for mp in range(MP):
            for t, (wa, rb) in enumerate(
                    terms(xn8h_t[mp], xn8l_t[mp], p8h_t[mp], p8l_t[mp])):
                nc.tensor.matmul(
                    u, wa[:, :, fo * P:(fo + 1) * P],
                    rb[:, :, hh * H:(hh + 1) * H],
                    start=(mp == 0 and t == 0),
                    stop=(mp == MP - 1 and t == 2), perf_mode=DR)
        hs = U8h_t[fo // 2][:, fo % 2, hh * H:(hh + 1) * H]
        nc.scalar.activation(hs, u, Copy)
        (lo_eng or nc.vector).tensor_tensor(
            out=U8l_t[fo // 2][:, fo % 2, hh * H:(hh + 1) * H],
            in0=u, in1=hs, op=Sub)

    def r8_sums(hh, mp, sumsp):
        """rowsums of p8 (hi+lo) for query blocks of half hh."""
        for j, px in enumerate((p8h_t, p8l_t)):
            sm = sumsp.tile([P, 4], F32, name="r8_m", tag="sums_m")
            for q in range(4):
                nc.tensor.matmul(
                    sm[:, q:q + 1],
                    px[mp][:, :, hh * H + q * P: hh * H + (q + 1) * P],
                    ones8, start=True, stop=True, perf_mode=DR)
            if mp == 0 and j == 0:
                nc.vector.tensor_copy(r8_acc[hh], sm)
            else:
                nc.vector.tensor_tensor(out=r8_acc[hh], in0=r8_acc[hh],
                                        in1=sm, op=Add)

    def recf_path(hh):
        r8s = tmpp.tile([P, 4], F32, name="r8s", tag="rf")
        nc.vector.tensor_scalar_mul(r8s, r8_acc[hh], RECF_SCALE)
        nc.vector.reciprocal(recf[hh], r8s)

    def y_chain(qt, he, ypp, ysp):
        hh = qt // 4
        yps = ypp.tile([P, H], F32, name="yps", tag="chain")
        for fp in range(FP):
            for t, (wa, rb) in enumerate(
                    terms(U8h_t[fp], U8l_t[fp], w28h_t[fp], w28l_t[fp])):
                nc.tensor.matmul(
                    yps, wa[:, :, qt * P:(qt + 1) * P],
                    rb[:, :, he * H:(he + 1) * H],
                    start=(fp == 0 and t == 0),
                    stop=(fp == FP - 1 and t == 2), perf_mode=DR)
        ysb = ysp.tile([P, H], BF16, name="ysb", tag="ysb")
        npieces = 2 if (qt, he) in ((6, 1), (7, 0), (7, 1)) else 1
        w = H // npieces
        for pc in range(npieces):
            sl = slice(pc * w, (pc + 1) * w)
            if (qt, he, pc) == (7, 1, 1):
                # final piece: DVE-only path so it runs beside ACT's piece 0
                rb = recf[hh][:, (qt % 4):(qt % 4) + 1].broadcast_to((P, w))
                nc.vector.tensor_tensor(out=ysb[:, sl], in0=yps[:, sl],
                                        in1=rb, op=Mul)
            else:
                nc.scalar.activation(ysb[:, sl], yps[:, sl], Copy,
                                     scale=recf[hh][:, (qt % 4):(qt % 4) + 1])
            nc.vector.tensor_tensor(out=ysb[:, sl], in0=ysb[:, sl],
                                    in1=bo_b[:, he * H + pc * w:
                                             he * H + (pc + 1) * w], op=Add)
            if (qt, he) == (7, 1):
                eng = nc.sync
            elif qt >= 6:
                eng = (nc.gpsimd, nc.sync)[(2 * he + pc) % 2]
            else:
                eng = (nc.gpsimd, nc.sync, nc.scalar)[(2 * he + pc + qt) % 3]
            eng.dma_start(
                out=y[qt * P:(qt + 1) * P,
                      he * H + pc * w: he * H + (pc + 1) * w],
                in_=ysb[:, sl])

    with tc.tile_pool(name="smps", bufs=2, space="PSUM") as sumsp, \
         tc.tile_pool(name="workp", bufs=6, space="PSUM") as workp:
        rtpp = sumsp
        # ---- PE pstate warmup while first loads land ----
        wps = workp.tile([P, 256], F32, name="wps", tag="chain")
        for i in range(9):
            nc.tensor.matmul(wps, warm[:, 0:P], warm, start=True, stop=True)
        # ---- T: 4 groups of 4 chains, fp-outer interleaved per group ----
        for fos, h in ((range(0, 4), 0), (range(0, 4), 1),
                       (range(4, 8), 0), (range(4, 8), 1)):
            tps = {fo: workp.tile([P, H], F32, name=f"t{fo}_{h}", tag="chain")
                   for fo in fos}
            g = fos[0] // 4
            for fp in range(FP):
                ah, al = a8q[fp][2 * g], a8q[fp][2 * g + 1]
                qh, ql = xqq[fp][2 * h], xqq[fp][2 * h + 1]
                for t, (wa, rb) in enumerate(terms(ah, al, qh, ql)):
                    for fo in fos:
                        c = fo % 4
                        nc.tensor.matmul(
                            tps[fo], wa[:, :, c * P:(c + 1) * P], rb,
                            start=(fp == 0 and t == 0),
                            stop=(fp == FP - 1 and t == 2), perf_mode=DR)
            for fo in fos:
                hs = T8h_t[fo // 2][:, fo % 2, h * H:(h + 1) * H]
                nc.scalar.activation(hs, tps[fo], Copy)
                nc.vector.tensor_tensor(
                    out=T8l_t[fo // 2][:, fo % 2, h * H:(h + 1) * H],
                    in0=tps[fo], in1=hs, op=Sub)
        pa.release()
        ysp = tc.alloc_tile_pool(name="ysb_p", bufs=6, side="right")

        # ---- S half 0 ----
        for mt in range(MT):
            s_chain(0, mt, workp)
            row_sums(0, mt, sumsp)
        # ---- S half 1, h0 softmax pipeline interleaved ----
        s_chain(1, 0, workp)
        row_sums(1, 0, sumsp)
        r_path(0, rtpp)
        for mt in range(1, MT):
            s_chain(1, mt, workp)
            row_sums(1, mt, sumsp)
            if mt % 2 == 0:
                p8_prod(0, mt // 2 - 1)
        p8_prod(0, MP - 1)
        # ---- U half 0; h1 softmax pipeline + h0 fp8 rowsums ----
        r_path(1, rtpp)
        u_chain(0, 0, workp)
        for fo in range(1, 8):
            p8_prod(1, fo - 1, lo_eng=nc.gpsimd if fo <= 7 else nc.vector)
            u_chain(0, fo, workp)
            r8_sums(0, fo - 1, sumsp)
        p8_prod(1, MP - 1, lo_eng=nc.vector)
        r8_sums(0, MP - 1, sumsp)
        recf_path(0)
        # ---- U half 1; h1 fp8 rowsums ----
        for fo in range(8):
            u_chain(1, fo, workp)
            if fo < MP:
                r8_sums(1, fo, sumsp)
        recf_path(1)
        # ---- Y ----
        for qt in (0, 1, 2, 3):
            for he in range(2):
                y_chain(qt, he, workp, ysp)
        for qt in (4, 5, 6):
            for he in range(2):
                y_chain(qt, he, workp, ysp)
        y_chain(7, 0, workp, ysp)
        for cb in range(2):
            # final chain split into two half-width chains so only a
            # 256-col post-pipeline trails the last matmul
            yw = workp.tile([P, H // 2], F32, name="ywf", tag="chain")
            for fp in range(FP):
                for t, (wa, rb) in enumerate(
                        terms(U8h_t[fp], U8l_t[fp], w28h_t[fp], w28l_t[fp])):
                    nc.tensor.matmul(
                        yw, wa[:, :, 7 * P:8 * P],
                        rb[:, :, H + cb * (H // 2):H + (cb + 1) * (H // 2)],
                        start=(fp == 0 and t == 0),
                        stop=(fp == FP - 1 and t == 2), perf_mode=DR)
            ysbf = ysp.tile([P, H // 2], BF16, name="ysbf", tag="ysb")
            nc.scalar.activation(ysbf, yw, Copy, scale=recf[1][:, 3:4])
            nc.vector.tensor_tensor(
                out=ysbf, in0=ysbf,
                in1=bo_b[:, H + cb * (H // 2):H + (cb + 1) * (H // 2)],
                op=Add)
            eng = nc.gpsimd if cb == 0 else nc.sync
            eng.dma_start(
                out=y[7 * P:8 * P, H + cb * (H // 2):H + (cb + 1) * (H // 2)],
                in_=ysbf)

    ysp.release()
    tmpp.release()
    smp.release()
    u8p.release()
    p8p.release()
    pres.release()
    t8p.release()
    cst.release()


_NC_CACHE = None


def _get_program():
    global _NC_CACHE
    if _NC_CACHE is None:
        _NC_CACHE = build_program()
    return _NC_CACHE


def _split8(v):
    hi = v.astype(E4NP)
    lo = (v - hi.astype(np.float32)).astype(E4NP)
    return hi, lo


def _pack_fp(mat, cols):
    """[E, cols] -> [FP, P, 2, cols] pair-packed along the contraction dim."""
    return np.ascontiguousarray(
        mat.reshape(FP, 2, P, cols).transpose(0, 2, 1, 3))


def _pack_hl(hi, lo, nt, cols):
    """pair-packed hi/lo [nt, P, 2, cols] -> [nt, P, 4*cols] row-concat."""
    out = np.empty((nt, P, 4 * cols), dtype=hi.dtype)
    out[:, :, 0:2 * cols] = hi.reshape(nt, P, 2 * cols)
    out[:, :, 2 * cols:] = lo.reshape(nt, P, 2 * cols)
    return out


def _pack_quad(hi, lo, nt, cols):
    """[nt, P, 2, cols] hi/lo -> [nt, P, 4*cols] as hiA|loA|hiB|loB,
    where A = cols[0:cols//2], B = cols[cols//2:]."""
    hw = cols // 2
    out = np.empty((nt, P, 4 * cols), dtype=hi.dtype)
    out[:, :, 0 * cols:1 * cols] = hi[:, :, :, 0:hw].reshape(nt, P, cols)
    out[:, :, 1 * cols:2 * cols] = lo[:, :, :, 0:hw].reshape(nt, P, cols)
    out[:, :, 2 * cols:3 * cols] = hi[:, :, :, hw:].reshape(nt, P, cols)
    out[:, :, 3 * cols:4 * cols] = lo[:, :, :, hw:].reshape(nt, P, cols)
    return out


def _host_prep(x, W_qkv, W_out, b_out):
    Wq = W_qkv[:, :E].astype(np.float64)
    Wk = W_qkv[:, E:2 * E].astype(np.float64)
    Wv = W_qkv[:, 2 * E:].astype(np.float64)
    A = ((Wq @ Wk.T) * (0.125 * 128.0)).astype(np.float32)
    W2 = ((Wv @ W_out.astype(np.float64)) * 16.0).astype(np.float32)
    A8h, A8l = _split8(A)
    W28h, W28l = _split8(W2)
    a8_p = _pack_quad(_pack_fp(A8h, E), _pack_fp(A8l, E), FP, E)
    w28_p = _pack_hl(_pack_fp(W28h, E), _pack_fp(W28l, E), FP, E)
    bo = b_out.astype(bfloat16)
    identity = np.eye(P, dtype=bfloat16)

    in_maps = []
    for c in range(8):
        b, half = divmod(c, 2)
        xb = np.asarray(x[b], dtype=np.float32)
        xrot = np.concatenate([xb[half * NQ:], xb[:half * NQ]], axis=0)
        x8h, x8l = _split8(xrot)
        xt8h = _pack_fp(np.ascontiguousarray(x8h.T), N)
        xt8l = _pack_fp(np.ascontiguousarray(x8l.T), N)
        xn8h = np.ascontiguousarray(
            x8h.reshape(MP, 2, P, E).transpose(0, 2, 1, 3))
        xn8l = np.ascontiguousarray(
            x8l.reshape(MP, 2, P, E).transpose(0, 2, 1, 3))
        in_maps.append({
            "a8p": a8_p,
            "xq8p": _pack_quad(xt8h[:, :, :, 0:NQ], xt8l[:, :, :, 0:NQ],
                               FP, NQ),
            "xk8p": _pack_quad(xt8h[:, :, :, NQ:N], xt8l[:, :, :, NQ:N],
                               FP, NQ),
            "xn8p": _pack_hl(xn8h, xn8l, MP, E),
            "w28p": w28_p,
            "bout": bo, "ident": identity,
        })
    return in_maps


def kernel(x, W_qkv, W_out, b_out):
    from concourse.bass_utils import run_bass_kernel_spmd

    x = np.asarray(x, dtype=np.float32)
    W_qkv = np.asarray(W_qkv, dtype=np.float32)
    W_out = np.asarray(W_out, dtype=np.float32)
    b_out = np.asarray(b_out, dtype=np.float32)

    nc = _get_program()
    in_maps = _host_prep(x, W_qkv, W_out, b_out)
    res = run_bass_kernel_spmd(nc, in_maps, list(range(8)))
    out = np.empty((B, N, E), dtype=np.float32)
    for c in range(8):
        b, half = divmod(c, 2)
        out[b, half * NQ:(half + 1) * NQ] = res.results[c]["y"].astype(
            np.float32)
    return out
